# revision 1
# baseline (speedup 1.0000x reference)
"""Trainium2 Bass kernel for nn_BDH_4406636445711 (dense transformer).

Sharding: 8 cores = data-parallel over B(2) x tensor-parallel over H(4).
Core c handles (b = c//4, h = c%4): its head's Dx/Dy slices, E rows, and a
V/4 shard of the readout. Per layer the y@E partial is AllReduced within
each b-group of 4 cores; v stays replicated inside the group. The host
stitches the 8 per-core [VS, T] logit shards into the full [B, T, V].

Matmuls run in float32r (full-rate fp32 with 11-bit RNE mantissa rounding,
~1.2e-4 relative noise vs fp32's 4x slower exact mode). v is kept
transposed ([D, T] "dT layout") as the primary representation; LayerNorms
over D (the partition dim) use ones-vector matmul column sums plus PE
rank-1 broadcast of the per-token -mean/rstd back to [128, T] tiles.
"""

import os
import sys

sys.path.insert(0, "/opt/trn_rl_repo")

import numpy as np

import concourse.bass as bass
import concourse.tile as tile
from concourse import bacc, mybir
from concourse.bass_utils import run_bass_kernel_spmd
from concourse.masks import make_identity
from concourse import library_config

F32 = mybir.dt.float32
F32R = mybir.dt.float32r
I32 = mybir.dt.int32
AF = mybir.ActivationFunctionType
OP = mybir.AluOpType

B, T, H, D, K, V, L = 2, 2048, 4, 256, 1024, 32000, 6
VS = V // 4          # vocab shard per core within a b-group
EPS = 1e-5
NT = T // 128        # 16 token tiles
NKT = K // 128       # 8 k' tiles
ND = D // 128        # 2 d tiles
TH = T // 2          # t-half = 1024
NS = TH // 512       # 512-wide matmul chunks per t-half

N_LAYERS = int(os.environ.get("KRN_LAYERS", str(L)))
DO_READOUT = os.environ.get("KRN_READOUT", "1") == "1"


def build(nc):
    # ---- DRAM parameters (per core) ----
    tok_d = nc.dram_tensor("tok", [T], I32, kind="ExternalInput")
    emb_d = nc.dram_tensor("emb", [V, D], F32, kind="ExternalInput")
    posT_d = nc.dram_tensor("posT", [D, T], F32, kind="ExternalInput")
    dx_d = nc.dram_tensor("dx", [D, K], F32R, kind="ExternalInput")
    dy_d = nc.dram_tensor("dy", [D, K], F32R, kind="ExternalInput")
    e_d = nc.dram_tensor("eh", [K, D], F32R, kind="ExternalInput")
    ro_d = nc.dram_tensor("ro", [D, VS], F32R, kind="ExternalInput")
    cos_d = nc.dram_tensor("cosh", [4, 128, T], F32, kind="ExternalInput")
    sin_d = nc.dram_tensor("sinh", [4, 128, T], F32, kind="ExternalInput")
    out_d = nc.dram_tensor("logitsT", [VS, T], F32, kind="ExternalOutput")
    vdbg_d = nc.dram_tensor("vdbg", [ND, 128, T], F32, kind="ExternalOutput")

    groups = [[0, 1, 2, 3], [4, 5, 6, 7]]

    with tile.TileContext(nc) as tc:
        with (
            nc.allow_low_precision(reason="float32r rounding is intentional"),
            tc.tile_pool(name="persist", bufs=1) as pp,
            tc.tile_pool(name="w8", bufs=6) as w8p,
            tc.tile_pool(name="t4", bufs=6) as t4p,
            tc.tile_pool(name="stats", bufs=3) as stp,
            tc.tile_pool(name="psmm", bufs=2, space="PSUM") as psmm,
            tc.tile_pool(name="psacc", bufs=1, space="PSUM") as psacc,
            tc.tile_pool(name="dram", bufs=1, space="DRAM") as dpool,
        ):
            _ctr = [0]

            def _nm(p):
                _ctr[0] += 1
                return f"{p}{_ctr[0]}"

            def w8(dt=F32):
                return w8p.tile([128, T], dt, tag="w8", name=_nm("w8_"))

            def w8n(dt=F32):
                return w8p.tile([128, ND, TH], dt, tag="w8", name=_nm("w8n_"))

            def t4(dt=F32):
                return t4p.tile([128, TH], dt, tag="t4", name=_nm("t4_"))

            def pmm(shape=None, dt=F32):
                return psmm.tile(shape or [128, TH], dt, tag="mm", name=_nm("mm_"))

            # ---- constants ----
            ident_f = t4p.tile([128, 128], F32, tag="t4", name="identf")
            make_identity(nc, ident_f[:])
            ident_r = pp.tile([128, 128], F32R)
            nc.vector.tensor_copy(ident_r[:], ident_f[:])
            ones_pf = pp.tile([128, 1], F32)
            nc.vector.memset(ones_pf[:], 1.0)
            ones_p = pp.tile([128, 1], F32R)
            nc.vector.tensor_copy(ones_p[:], ones_pf[:])
            ones_cf = pp.tile([1, 128], F32)
            nc.vector.memset(ones_cf[:], 1.0)
            ones_c = pp.tile([1, 128], F32R)
            nc.vector.tensor_copy(ones_c[:], ones_cf[:])
            eps_p = pp.tile([128, 1], F32)
            nc.vector.memset(eps_p[:], EPS)
            eps_1 = pp.tile([1, 1], F32)
            nc.vector.memset(eps_1[:], EPS)
            nc.gpsimd.load_library(library_config.attn)

            # ---- persistent tensors ----
            vT = pp.tile([128, ND, T], F32R)          # v (dT layout), updated in place
            qT = pp.tile([128, NKT, T], F32R)
            vp_td = pp.tile([128, NT, D], F32R)       # (v+pos) in td layout
            dx_sb = pp.tile([128, ND, K], F32R)
            nc.sync.dma_start(dx_sb[:], dx_d.ap().rearrange("(c p) k -> p c k", p=128))
            dy_sb = pp.tile([128, ND, K], F32R)
            nc.sync.dma_start(dy_sb[:], dy_d.ap().rearrange("(c p) k -> p c k", p=128))
            e_sb = pp.tile([128, NKT, D], F32R)
            nc.sync.dma_start(e_sb[:], e_d.ap().rearrange("(c p) d -> p c d", p=128))

            # ---- internal DRAM ----
            xspill = dpool.tile([NKT, 128, T], F32, tag="xspill")
            cc_in = [dpool.tile([ND, 128, TH], F32, tag=f"cci{i}", name=f"cci{i}")
                     for i in range(2)]
            cc_out = [dpool.tile([ND, 128, TH], F32, tag=f"cco{i}", name=f"cco{i}")
                      for i in range(2)]

            def pstats(negmean_src_ps, s2_src_ps, n):
                """negmean=-s1/n, rstd=1/sqrt(s2/n-mean^2+eps) as [1, TH] f32r."""
                negmean = stp.tile([1, TH], F32R, tag="st", name=_nm("st_"))
                nc.vector.tensor_scalar_mul(negmean[:], negmean_src_ps, -1.0 / n)
                m2 = stp.tile([1, TH], F32, tag="st", name=_nm("st_"))
                nc.vector.tensor_mul(m2[:], negmean[:].bitcast(F32),
                                     negmean[:].bitcast(F32))
                var = stp.tile([1, TH], F32, tag="st", name=_nm("st_"))
                nc.vector.scalar_tensor_tensor(
                    out=var[:], in0=s2_src_ps, scalar=1.0 / n, in1=m2[:],
                    op0=OP.mult, op1=OP.subtract)
                lnv = stp.tile([1, TH], F32, tag="st", name=_nm("st_"))
                nc.scalar.activation(lnv[:], var[:], AF.Ln, bias=eps_1[:])
                rstd = stp.tile([1, TH], F32R, tag="st", name=_nm("st_"))
                nc.scalar.activation(rstd[:], lnv[:], AF.Exp, scale=-0.5)
                return negmean, rstd

            def colsums(src, t0):
                """s1[t]=sum_d src[d,t], s2[t]=sum_d src[d,t]^2 over ND tiles.

                src is a [128, ND, T]-like f32r AP ([d-part, dc, t]); returns
                two [1, TH] psum tiles for the t-half starting at t0."""
                s1 = pmm()
                for dc in range(ND):
                    for ns in range(NS):
                        nc.tensor.matmul(
                            s1[:1, ns * 512:(ns + 1) * 512], ones_p[:],
                            src[:, dc, t0 + ns * 512:t0 + (ns + 1) * 512],
                            start=(dc == 0), stop=(dc == ND - 1),
                            skip_group_check=True)
                sq = w8n(F32R)
                for dc in range(ND):
                    nc.scalar.activation(sq[:, dc],
                                         src[:, dc, t0:t0 + TH].bitcast(F32),
                                         AF.Square)
                s2 = pmm()
                for dc in range(ND):
                    for ns in range(NS):
                        nc.tensor.matmul(
                            s2[:1, ns * 512:(ns + 1) * 512], ones_p[:],
                            sq[:, dc, ns * 512:(ns + 1) * 512],
                            start=(dc == 0), stop=(dc == ND - 1),
                            skip_group_check=True)
                return s1[:1, :], s2[:1, :]

            def bcast(vec):
                """PE rank-1 broadcast of a [1, TH] f32r vector to [128, TH] psum."""
                out = pmm()
                for ns in range(NS):
                    nc.tensor.matmul(out[:, ns * 512:(ns + 1) * 512], ones_c[:],
                                     vec[:, ns * 512:(ns + 1) * 512],
                                     start=True, stop=True)
                return out

            # ============ embedding gather + LN -> v0 -> transpose to vT ============
            idx = pp.tile([128, NT], I32)
            nc.sync.dma_start(idx[:], tok_d.ap().rearrange("(n p) -> p n", p=128))
            for n in range(NT):
                gat = t4p.tile([128, D], F32, tag="t4")
                nc.gpsimd.indirect_dma_start(
                    out=gat[:], out_offset=None, in_=emb_d.ap(),
                    in_offset=bass.IndirectOffsetOnAxis(ap=idx[:, n:n + 1], axis=0),
                )
                stats = t4p.tile([128, 6], F32, tag="t4")
                nc.vector.bn_stats(out=stats[:], in_=gat[:])
                mv = t4p.tile([128, 2], F32, tag="t4")
                nc.vector.bn_aggr(out=mv[:], in_=stats[:])
                std = t4p.tile([128, 1], F32, tag="t4")
                nc.scalar.activation(std[:], mv[:, 1:2], AF.Sqrt, bias=eps_p[:])
                rstd = t4p.tile([128, 1], F32, tag="t4")
                nc.vector.reciprocal(rstd[:], std[:])
                v0 = t4p.tile([128, D], F32R, tag="t4")
                nc.vector.tensor_scalar(
                    out=v0[:], in0=gat[:], scalar1=mv[:, 0:1], scalar2=rstd[:],
                    op0=OP.subtract, op1=OP.mult)
                for dc in range(ND):
                    tp = pmm([128, 128], F32R)
                    nc.tensor.transpose(out=tp[:], in_=v0[:, dc * 128:(dc + 1) * 128],
                                        identity=ident_r[:])
                    nc.vector.tensor_copy(vT[:, dc, n * 128:(n + 1) * 128], tp[:])

            def phaseA(th):
                """v[:, th-half] += pos; transpose that half into vp_td."""
                for dc in range(ND):
                    pch = t4()
                    nc.sync.dma_start(
                        pch[:], posT_d.ap()[dc * 128:(dc + 1) * 128,
                                            th * TH:(th + 1) * TH])
                    nc.vector.tensor_add(
                        vT[:, dc, th * TH:(th + 1) * TH],
                        vT[:, dc, th * TH:(th + 1) * TH].bitcast(F32),
                        pch[:])
                for dc in range(ND):
                    tp = pmm([128, 8, 128], F32R)
                    for k in range(8):
                        n = th * 8 + k
                        nc.tensor.transpose(out=tp[:, k, :],
                                            in_=vT[:, dc, n * 128:(n + 1) * 128],
                                            identity=ident_r[:])
                    nc.vector.tensor_copy(
                        vp_td[:, th * 8:(th + 1) * 8,
                              dc * 128:(dc + 1) * 128], tp[:])

            # ================================ layers ================================
            for layer in range(N_LAYERS):
                # ---- A (layer 0 only; later layers fold A into E bodies) ----
                if layer == 0:
                    for th in range(2):
                        phaseA(th)

                # ---- B: x = relu(v @ Dx) (kT layout); RoPE -> q; spill x ----
                for i in range(4):
                    cos_t = w8()
                    nc.sync.dma_start(cos_t[:], cos_d.ap()[i])
                    sin_t = w8()
                    nc.sync.dma_start(sin_t[:], sin_d.ap()[i])
                    xts = {}
                    for ii in (i, i + 4):
                        xt = w8()
                        xts[ii] = xt
                        for th in range(2):
                            px = pmm()
                            for dc in range(ND):
                                for ns in range(NS):
                                    nc.tensor.matmul(
                                        px[:, ns * 512:(ns + 1) * 512],
                                        dx_sb[:, dc, ii * 128:(ii + 1) * 128],
                                        vT[:, dc, th * TH + ns * 512:
                                           th * TH + (ns + 1) * 512],
                                        start=(dc == 0), stop=(dc == ND - 1))
                            nc.scalar.activation(xt[:, th * TH:(th + 1) * TH],
                                                 px[:], AF.Relu)
                        nc.sync.dma_start(xspill[ii], xt[:])
                    xi, xj = xts[i], xts[i + 4]
                    m1 = w8()
                    nc.vector.tensor_mul(m1[:], xi[:], cos_t[:])
                    m2 = w8()
                    nc.vector.tensor_mul(m2[:], xj[:], sin_t[:])
                    nc.vector.tensor_sub(qT[:, i], m1[:], m2[:])
                    m3 = w8()
                    nc.vector.tensor_mul(m3[:], xj[:], cos_t[:])
                    m4 = w8()
                    nc.vector.tensor_mul(m4[:], xi[:], sin_t[:])
                    nc.vector.tensor_add(qT[:, i + 4], m3[:], m4[:])

                # ---- C+D per t-half ----
                for th in range(2):
                    t0 = th * TH
                    # C: scores[s_j, t-half] -> aT += vp_td[s_j]^T @ scores
                    pa = psacc.tile([128, ND, TH], F32, tag="acc")
                    for j in range(NT):
                        pscr = pmm()
                        for kc in range(NKT):
                            for ns in range(NS):
                                nc.tensor.matmul(
                                    pscr[:, ns * 512:(ns + 1) * 512],
                                    qT[:, kc, j * 128:(j + 1) * 128],
                                    qT[:, kc, t0 + ns * 512:t0 + (ns + 1) * 512],
                                    start=(kc == 0), stop=(kc == NKT - 1))
                        scr = t4(F32R)
                        nc.vector.tensor_copy(scr[:], pscr[:])
                        for dc in range(ND):
                            for ns in range(NS):
                                nc.tensor.matmul(
                                    pa[:, dc, ns * 512:(ns + 1) * 512],
                                    vp_td[:, j, dc * 128:(dc + 1) * 128],
                                    scr[:, ns * 512:(ns + 1) * 512],
                                    start=(j == 0), stop=(j == NT - 1),
                                    skip_group_check=True)
                    aT = w8n(F32R)
                    for dc in range(ND):
                        nc.vector.tensor_copy(aT[:, dc], pa[:, dc])
                    # ln(a) over d (partition dim)
                    s1, s2 = colsums(aT, 0)
                    negmean, rstd = pstats(s1, s2, D)
                    nm_b = bcast(negmean)
                    rs_b = bcast(rstd)
                    lnA = w8n(F32R)
                    for dc in range(ND):
                        cent = t4()
                        nc.vector.tensor_add(cent[:], aT[:, dc].bitcast(F32), nm_b[:])
                        nc.vector.tensor_mul(lnA[:, dc], cent[:], rs_b[:])

                    # D: y_i = relu(lnA @ Dy_i) * x_i ; yET += E_i^T-style accum
                    pye = psacc.tile([128, ND, TH], F32, tag="acc")
                    for i in range(NKT):
                        py = pmm()
                        for dc in range(ND):
                            for ns in range(NS):
                                nc.tensor.matmul(
                                    py[:, ns * 512:(ns + 1) * 512],
                                    dy_sb[:, dc, i * 128:(i + 1) * 128],
                                    lnA[:, dc, ns * 512:(ns + 1) * 512],
                                    start=(dc == 0), stop=(dc == ND - 1))
                        xr = t4()
                        nc.sync.dma_start(xr[:], xspill[i, :, t0:t0 + TH])
                        yt = t4(F32R)
                        nc.vector.scalar_tensor_tensor(
                            out=yt[:], in0=py[:], scalar=0.0, in1=xr[:],
                            op0=OP.max, op1=OP.mult)
                        for dc in range(ND):
                            for ns in range(NS):
                                nc.tensor.matmul(
                                    pye[:, dc, ns * 512:(ns + 1) * 512],
                                    e_sb[:, i, dc * 128:(dc + 1) * 128],
                                    yt[:, ns * 512:(ns + 1) * 512],
                                    start=(i == 0), stop=(i == NKT - 1),
                                    skip_group_check=True)
                    ye = w8n(F32)
                    for dc in range(ND):
                        nc.vector.tensor_copy(ye[:, dc], pye[:, dc])
                        nc.sync.dma_start(cc_in[th][:][dc], ye[:, dc])
                    nc.gpsimd.collective_compute(
                        "AllReduce", OP.add, replica_groups=groups,
                        ins=[cc_in[th][:].opt()], outs=[cc_out[th][:].opt()])

                # ---- E: u = ln(sum); w = vp + u; v = ln(w) (dT layout) ----
                for th in range(2):
                    t0 = th * TH
                    uT = w8n(F32R)
                    for dc in range(ND):
                        nc.sync.dma_start(uT[:, dc].bitcast(F32), cc_out[th][:][dc])
                    s1, s2 = colsums(uT, 0)
                    negmean, rstd = pstats(s1, s2, D)
                    nm_b = bcast(negmean)
                    rs_b = bcast(rstd)
                    for dc in range(ND):
                        cent = t4()
                        nc.vector.tensor_add(cent[:], uT[:, dc].bitcast(F32), nm_b[:])
                        lnu = t4()
                        nc.vector.tensor_mul(lnu[:], cent[:], rs_b[:])
                        nc.vector.tensor_add(vT[:, dc, t0:t0 + TH],
                                             vT[:, dc, t0:t0 + TH].bitcast(F32),
                                             lnu[:])
                    s1, s2 = colsums(vT, t0)
                    negmean, rstd = pstats(s1, s2, D)
                    nm_b = bcast(negmean)
                    rs_b = bcast(rstd)
                    for dc in range(ND):
                        cent = t4()
                        nc.vector.tensor_add(cent[:],
                                             vT[:, dc, t0:t0 + TH].bitcast(F32),
                                             nm_b[:])
                        nc.vector.tensor_mul(vT[:, dc, t0:t0 + TH],
                                             cent[:], rs_b[:])
                    if layer < N_LAYERS - 1:
                        phaseA(th)

            # debug dump of final vT
            for dc in range(ND):
                nc.sync.dma_start(vdbg_d.ap()[dc], vT[:, dc].bitcast(F32))

            # ============= readout: logitsT = (v @ readout)^T, V-sharded =============
            if DO_READOUT:
                nvb = (VS + 127) // 128
                for vb in range(nvb):
                    m = min(128, VS - vb * 128)
                    ro_sb = t4p.tile([128, ND, 128], F32R, tag="ro",
                                     bufs=2, name=_nm("ro_"))
                    for dc in range(ND):
                        nc.sync.dma_start(
                            ro_sb[:, dc, :m],
                            ro_d.ap()[dc * 128:(dc + 1) * 128,
                                      vb * 128:vb * 128 + m])
                    lo = w8()
                    for th in range(2):
                        pl = pmm()
                        for dc in range(ND):
                            for ns in range(NS):
                                nc.tensor.matmul(
                                    pl[:m, ns * 512:(ns + 1) * 512],
                                    ro_sb[:, dc, :m],
                                    vT[:, dc, th * TH + ns * 512:
                                       th * TH + (ns + 1) * 512],
                                    start=(dc == 0), stop=(dc == ND - 1))
                        nc.scalar.copy(lo[:m, th * TH:(th + 1) * TH], pl[:m])
                    nc.sync.dma_start(out_d.ap()[vb * 128:vb * 128 + m, :], lo[:m])

    nc.compile()
    return nc


_NC_CACHE = None


def _get_nc():
    global _NC_CACHE
    if _NC_CACHE is None:
        nc = bacc.Bacc("TRN2", target_bir_lowering=False, debug=False, num_devices=8)
        _NC_CACHE = build(nc)
    return _NC_CACHE


def _rope_tables():
    # match the jax reference: float32 angle computation
    inv_freq = (1.0 / (10000.0 ** (np.arange(0, K, 2, dtype=np.float32)
                                   / np.float32(K)))).astype(np.float32)
    t = np.arange(T, dtype=np.float32)
    freqs = (t[:, None] * inv_freq[None, :]).astype(np.float32)  # [T, K/2]
    cos = np.cos(freqs).astype(np.float32)
    sin = np.sin(freqs).astype(np.float32)
    cosT = np.ascontiguousarray(cos.T).reshape(4, 128, T)
    sinT = np.ascontiguousarray(sin.T).reshape(4, 128, T)
    return cosT, sinT


def kernel(input_, emb, pos, Dx, Dy, E, readout):
    input_ = np.asarray(input_)
    emb = np.ascontiguousarray(np.asarray(emb, dtype=np.float32))
    pos = np.asarray(pos, dtype=np.float32)
    Dx = np.asarray(Dx, dtype=np.float32)
    Dy = np.asarray(Dy, dtype=np.float32)
    E = np.asarray(E, dtype=np.float32)
    readout = np.asarray(readout, dtype=np.float32)

    nc = _get_nc()
    cosT, sinT = _rope_tables()
    posT = np.ascontiguousarray(pos.T)

    in_maps = []
    for c in range(8):
        b, h = divmod(c, 4)
        in_maps.append({
            "tok": np.ascontiguousarray(input_[b].astype(np.int32)),
            "emb": emb,
            "posT": posT,
            "dx": np.ascontiguousarray(Dx[h]),
            "dy": np.ascontiguousarray(Dy[h]),
            "eh": np.ascontiguousarray(E[h * K:(h + 1) * K]),
            "ro": np.ascontiguousarray(readout[:, h * VS:(h + 1) * VS]),
            "cosh": cosT,
            "sinh": sinT,
        })
    trace = os.environ.get("KRN_TRACE", "0") == "1"
    res = run_bass_kernel_spmd(nc, in_maps, list(range(8)), trace=trace)
    out = np.empty((B, T, V), dtype=np.float32)
    for c in range(8):
        b, h = divmod(c, 4)
        out[b, :, h * VS:(h + 1) * VS] = res.results[c]["logitsT"].T
    kernel._last_results = res
    return out



# revision 2
# speedup vs baseline: 1.5470x; 1.5470x over previous
"""Trainium2 Bass kernel for nn_BDH_4406636445711 (dense transformer).

Sharding: 8 cores = data-parallel over B(2) x tensor-parallel over H(4).
Core c handles (b = c//4, h = c%4): its head's Dx/Dy slices, E rows, and a
V/4 shard of the readout. Per layer the y@E partial is AllReduced within
each b-group of 4 cores. The host stitches the 8 per-core [VS, T] logit
shards (bf16 on device, cast to fp32 host-side) into the full [B, T, V].

Key algebraic optimization vs the naive graph: scores = q @ q^T is only
ever used for a = scores @ v, so we compute a = q @ (q^T v) instead --
G = q^T v is [K, D]; this is ~5x fewer PE cycles than materializing the
[T, T] score matrix (K = T/2, D << T).

Layouts: v lives token-major ("td": [128 tok part, 16 chunk, 256 d]) so
every LayerNorm is a free-dim bn_stats reduction (no PE column-sum
machinery). x/q live kT ([128 k part, 8 chunk, 2048 t]); q is additionally
transposed to tk for the G matmul. All matmul operands are bf16 (full PE
rate); accumulation and v/LN stats stay fp32. The ACT engine only uses
{Relu, Copy, Sqrt, Identity} so its function table never reloads.
"""

import os
import sys

sys.path.insert(0, "/opt/trn_rl_repo")

import numpy as np

import concourse.bass as bass
import concourse.tile as tile
from concourse import bacc, mybir
from concourse.bass_utils import run_bass_kernel_spmd
from concourse.masks import make_identity
from concourse import library_config

F32 = mybir.dt.float32
BF16 = mybir.dt.bfloat16
I32 = mybir.dt.int32
AF = mybir.ActivationFunctionType
OP = mybir.AluOpType

B, T, H, D, K, V, L = 2, 2048, 4, 256, 1024, 32000, 6
VS = V // 4          # vocab shard per core within a b-group
EPS = 1e-5
NT = T // 128        # 16 token chunks
NKC = K // 128       # 8 k chunks
ND = D // 128        # 2 d chunks
TH = T // 2          # 1024

N_LAYERS = int(os.environ.get("KRN_LAYERS", str(L)))
DO_READOUT = os.environ.get("KRN_READOUT", "1") == "1"


def build(nc):
    # ---- DRAM parameters (per core) ----
    tok_d = nc.dram_tensor("tok", [T], I32, kind="ExternalInput")
    emb_d = nc.dram_tensor("emb", [V, D], F32, kind="ExternalInput")
    pos_d = nc.dram_tensor("pos", [T, D], F32, kind="ExternalInput")
    dx_d = nc.dram_tensor("dxb", [D, K], BF16, kind="ExternalInput")
    dy_d = nc.dram_tensor("dyb", [D, K], BF16, kind="ExternalInput")
    e_d = nc.dram_tensor("eb", [K, D], BF16, kind="ExternalInput")
    ro_d = nc.dram_tensor("rob", [D, VS], BF16, kind="ExternalInput")
    cos_d = nc.dram_tensor("cosb", [8, 128, TH], BF16, kind="ExternalInput")
    sin_d = nc.dram_tensor("sinb", [8, 128, TH], BF16, kind="ExternalInput")
    out_d = nc.dram_tensor("logitsT", [VS, T], BF16, kind="ExternalOutput")

    groups = [[0, 1, 2, 3], [4, 5, 6, 7]]

    with tile.TileContext(nc) as tc:
        with (
            nc.allow_low_precision(reason="bf16 matmul path is intentional"),
            tc.tile_pool(name="persist", bufs=1) as pp,
            tc.tile_pool(name="w4", bufs=8) as w4p,     # [128,1024] bf16 rope
            tc.tile_pool(name="sm", bufs=6) as smp,     # [128,256] f32 temps
            tc.tile_pool(name="lnt", bufs=3) as lntp,   # [128,256] bf16 lnA_td
            tc.tile_pool(name="stats", bufs=12) as stp, # [128,8] f32
            tc.tile_pool(name="rop", bufs=3) as rop,    # readout weights
            tc.tile_pool(name="lop", bufs=3) as lop,    # logit staging
            tc.tile_pool(name="pb", bufs=2, space="PSUM") as pbp,   # [128,1024] f32
            tc.tile_pool(name="pa", bufs=2, space="PSUM") as pap,   # [128,256] f32
            tc.tile_pool(name="pt", bufs=2, space="PSUM") as ptp,   # [128,8,128] bf16
            tc.tile_pool(name="dram", bufs=1, space="DRAM") as dpool,
        ):
            _ctr = [0]

            def _nm(p):
                _ctr[0] += 1
                return f"{p}{_ctr[0]}"

            # ---- constants ----
            ident_f = smp.tile([128, 128], F32, tag="sm", name="identf")
            make_identity(nc, ident_f[:])
            ident_bf = pp.tile([128, 128], BF16)
            nc.vector.tensor_copy(ident_bf[:], ident_f[:])
            eps_p = pp.tile([128, 1], F32)
            nc.vector.memset(eps_p[:], EPS)
            nc.gpsimd.load_library(library_config.attn)

            # ---- persistent tensors ----
            v_td = pp.tile([128, NT, D], F32)          # v (+pos), token-major
            vp_bf = pp.tile([128, NT, D], BF16)        # bf16 cast of v_td
            vpT = pp.tile([128, ND, T], BF16)          # v transposed (d-major)
            qT = pp.tile([128, NKC, T], BF16)          # q k-major; reused as yt
            qtk = pp.tile([128, NT, NKC, 128], BF16)   # q token-major
            x_bf = pp.tile([128, NKC, T], BF16)        # relu(v@Dx), k-major
            g_bf = pp.tile([128, NKC, D], BF16)        # G = q^T (v+pos)
            lnA_dT = pp.tile([128, ND, T], BF16)       # ln(a) d-major
            pos_sb = pp.tile([128, NT, D], F32)
            dx_sb = pp.tile([128, ND, K], BF16)
            dy_sb = pp.tile([128, ND, K], BF16)
            e_sb = pp.tile([128, NKC, D], BF16)

            nc.sync.dma_start(pos_sb[:], pos_d.ap().rearrange("(j p) d -> p j d", p=128))
            nc.sync.dma_start(dx_sb[:], dx_d.ap().rearrange("(c p) k -> p c k", p=128))
            nc.sync.dma_start(dy_sb[:], dy_d.ap().rearrange("(c p) k -> p c k", p=128))
            nc.sync.dma_start(e_sb[:], e_d.ap().rearrange("(c p) d -> p c d", p=128))

            # ---- internal DRAM (collective staging) ----
            cc_in = [dpool.tile([8, 128, D], F32, tag=f"cci{i}", name=f"cci{i}")
                     for i in range(2)]
            cc_out = [dpool.tile([8, 128, D], F32, tag=f"cco{i}", name=f"cco{i}")
                      for i in range(2)]

            def ln_stats(src_ap):
                """rstd and -mean*rstd ([128,1] f32) of a [128, D] td chunk."""
                st = stp.tile([128, 8], F32, tag="st", name=_nm("st_"))
                nc.vector.bn_stats(st[:, 0:6], src_ap)
                mv = stp.tile([128, 8], F32, tag="st", name=_nm("st_"))
                nc.vector.bn_aggr(mv[:, 0:2], st[:, 0:6])
                sd = stp.tile([128, 8], F32, tag="st", name=_nm("st_"))
                nc.scalar.activation(sd[:, 0:1], mv[:, 1:2], AF.Sqrt, bias=eps_p[:])
                rstd = stp.tile([128, 8], F32, tag="st", name=_nm("st_"))
                nc.vector.reciprocal(rstd[:, 0:1], sd[:, 0:1])
                nmr = stp.tile([128, 8], F32, tag="st", name=_nm("st_"))
                nc.vector.tensor_scalar(
                    out=nmr[:, 0:1], in0=mv[:, 0:1], scalar1=rstd[:, 0:1],
                    scalar2=-1.0, op0=OP.mult, op1=OP.mult)
                return rstd, nmr

            def apply_ln(dst_ap, src_ap, rstd, nmr):
                # dst = (src - mean) * rstd, on the ACT engine
                nc.scalar.activation(dst_ap, src_ap, AF.Identity,
                                     bias=nmr[:, 0:1], scale=rstd[:, 0:1])

            def vchunk_finish(j, src_ap, add_pos):
                """v_td[:,j] = ln(src) (+pos); refresh vp_bf and vpT."""
                rstd, nmr = ln_stats(src_ap)
                if add_pos:
                    tmp = smp.tile([128, D], F32, tag="sm", name=_nm("sm_"))
                    apply_ln(tmp[:], src_ap, rstd, nmr)
                    nc.vector.tensor_add(v_td[:, j], tmp[:], pos_sb[:, j])
                else:
                    apply_ln(v_td[:, j], src_ap, rstd, nmr)
                nc.scalar.copy(vp_bf[:, j], v_td[:, j])
                tp = ptp.tile([128, 8, 128], BF16, tag="pt", name=_nm("pt_"))
                for dc in range(ND):
                    nc.tensor.transpose(tp[:, dc], vp_bf[:, j, dc * 128:(dc + 1) * 128],
                                        ident_bf[:])
                nc.vector.tensor_copy(vpT[:, :, j * 128:(j + 1) * 128], tp[:, 0:2])

            # ================= embedding gather + LN + pos =================
            idx = pp.tile([128, NT], I32)
            nc.sync.dma_start(idx[:], tok_d.ap().rearrange("(n p) -> p n", p=128))
            for j in range(NT):
                gat = smp.tile([128, D], F32, tag="sm", name=_nm("sm_"))
                nc.gpsimd.indirect_dma_start(
                    out=gat[:], out_offset=None, in_=emb_d.ap(),
                    in_offset=bass.IndirectOffsetOnAxis(ap=idx[:, j:j + 1], axis=0),
                )
                vchunk_finish(j, gat[:], add_pos=True)

            def phaseB(th):
                """x[:, th cols] = relu(vp @ Dx); RoPE -> q; build qtk."""
                c0 = th * TH
                for i in range(NKC):
                    px = pbp.tile([128, TH], F32, tag="pb", name=_nm("pb_"))
                    for dc in range(ND):
                        for ns in range(2):
                            nc.tensor.matmul(
                                px[:, ns * 512:(ns + 1) * 512],
                                dx_sb[:, dc, i * 128:(i + 1) * 128],
                                vpT[:, dc, c0 + ns * 512:c0 + (ns + 1) * 512],
                                start=(dc == 0), stop=(dc == ND - 1))
                    nc.scalar.activation(x_bf[:, i, c0:c0 + TH], px[:], AF.Relu)
                for i in range(4):
                    cos_t = w4p.tile([128, TH], BF16, tag="w4", name=_nm("w4_"))
                    nc.sync.dma_start(cos_t[:], cos_d.ap()[i * 2 + th])
                    sin_t = w4p.tile([128, TH], BF16, tag="w4", name=_nm("w4_"))
                    nc.sync.dma_start(sin_t[:], sin_d.ap()[i * 2 + th])
                    xi = x_bf[:, i, c0:c0 + TH]
                    xj = x_bf[:, i + 4, c0:c0 + TH]
                    m1 = w4p.tile([128, TH], BF16, tag="w4", name=_nm("w4_"))
                    nc.vector.tensor_mul(m1[:], xi, cos_t[:])
                    m2 = w4p.tile([128, TH], BF16, tag="w4", name=_nm("w4_"))
                    nc.vector.tensor_mul(m2[:], xj, sin_t[:])
                    nc.vector.tensor_sub(qT[:, i, c0:c0 + TH], m1[:], m2[:])
                    m3 = w4p.tile([128, TH], BF16, tag="w4", name=_nm("w4_"))
                    nc.vector.tensor_mul(m3[:], xj, cos_t[:])
                    m4 = w4p.tile([128, TH], BF16, tag="w4", name=_nm("w4_"))
                    nc.vector.tensor_mul(m4[:], xi, sin_t[:])
                    nc.vector.tensor_add(qT[:, i + 4, c0:c0 + TH], m3[:], m4[:])
                for j in range(th * 8, th * 8 + 8):
                    tq = ptp.tile([128, 8, 128], BF16, tag="pt", name=_nm("pt_"))
                    for kc in range(NKC):
                        nc.tensor.transpose(tq[:, kc], qT[:, kc, j * 128:(j + 1) * 128],
                                            ident_bf[:])
                    nc.vector.tensor_copy(qtk[:, j], tq[:])

            def phaseC():
                """G = q^T vp; a = q G (token-major) with fused LN -> lnA_dT."""
                for kc in range(NKC):
                    pg = pap.tile([128, D], F32, tag="pa", name=_nm("pa_"))
                    for j in range(NT):
                        nc.tensor.matmul(pg[:], qtk[:, j, kc], vp_bf[:, j],
                                         start=(j == 0), stop=(j == NT - 1))
                    nc.scalar.copy(g_bf[:, kc], pg[:])
                for j in range(NT):
                    paa = pap.tile([128, D], F32, tag="pa", name=_nm("pa_"))
                    for kc in range(NKC):
                        nc.tensor.matmul(paa[:], qT[:, kc, j * 128:(j + 1) * 128],
                                         g_bf[:, kc],
                                         start=(kc == 0), stop=(kc == NKC - 1))
                    rstd, nmr = ln_stats(paa[:])
                    lnt = lntp.tile([128, D], BF16, tag="lnt", name=_nm("lnt_"))
                    apply_ln(lnt[:], paa[:], rstd, nmr)
                    tp = ptp.tile([128, 8, 128], BF16, tag="pt", name=_nm("pt_"))
                    for dc in range(ND):
                        nc.tensor.transpose(tp[:, dc], lnt[:, dc * 128:(dc + 1) * 128],
                                            ident_bf[:])
                    nc.vector.tensor_copy(lnA_dT[:, :, j * 128:(j + 1) * 128],
                                          tp[:, 0:2])

            def phaseD(th):
                """y = relu(lnA@Dy)*x; u = y@E (token-major); AllReduce u."""
                c0 = th * TH
                yt = qT  # q is dead after phase C; reuse its buffer for y*x
                for i in range(NKC):
                    py = pbp.tile([128, TH], F32, tag="pb", name=_nm("pb_"))
                    for dc in range(ND):
                        for ns in range(2):
                            nc.tensor.matmul(
                                py[:, ns * 512:(ns + 1) * 512],
                                dy_sb[:, dc, i * 128:(i + 1) * 128],
                                lnA_dT[:, dc, c0 + ns * 512:c0 + (ns + 1) * 512],
                                start=(dc == 0), stop=(dc == ND - 1))
                    nc.vector.scalar_tensor_tensor(
                        out=yt[:, i, c0:c0 + TH], in0=py[:], scalar=0.0,
                        in1=x_bf[:, i, c0:c0 + TH], op0=OP.max, op1=OP.mult)
                for j in range(th * 8, th * 8 + 8):
                    pu = pap.tile([128, D], F32, tag="pa", name=_nm("pa_"))
                    for i in range(NKC):
                        nc.tensor.matmul(pu[:], yt[:, i, j * 128:(j + 1) * 128],
                                         e_sb[:, i],
                                         start=(i == 0), stop=(i == NKC - 1))
                    us = smp.tile([128, D], F32, tag="sm", name=_nm("sm_"))
                    nc.scalar.copy(us[:], pu[:])
                    nc.sync.dma_start(cc_in[th][:][j - th * 8], us[:])
                nc.gpsimd.collective_compute(
                    "AllReduce", OP.add, replica_groups=groups,
                    ins=[cc_in[th][:].opt()], outs=[cc_out[th][:].opt()])

            def phaseE(th, layer):
                """v = ln(v + ln(allreduce(u))) (+pos); refresh vp/vpT."""
                last = layer == N_LAYERS - 1
                for j in range(th * 8, th * 8 + 8):
                    us = smp.tile([128, D], F32, tag="sm", name=_nm("sm_"))
                    nc.sync.dma_start(us[:], cc_out[th][:][j - th * 8])
                    rstd, nmr = ln_stats(us[:])
                    uln = smp.tile([128, D], F32, tag="sm", name=_nm("sm_"))
                    apply_ln(uln[:], us[:], rstd, nmr)
                    nc.vector.tensor_add(v_td[:, j], v_td[:, j], uln[:])
                    vchunk_finish(j, v_td[:, j], add_pos=not last)

            # ================================ layers ================================
            phaseB(0)
            phaseB(1)
            for layer in range(N_LAYERS):
                with nc.named_scope(f"L{layer}"):
                    phaseC()
                    phaseD(0)
                    phaseD(1)
                    phaseE(0, layer)
                    if layer < N_LAYERS - 1:
                        phaseB(0)
                    phaseE(1, layer)
                    if layer < N_LAYERS - 1:
                        phaseB(1)

            # ============= readout: logitsT = (v @ readout)^T, V-sharded =============
            if DO_READOUT:
                nvb = (VS + 127) // 128
                with nc.named_scope("readout"):
                    for vb in range(nvb):
                        m = min(128, VS - vb * 128)
                        rot = rop.tile([128, ND, 128], BF16, tag="ro", name=_nm("ro_"))
                        for dc in range(ND):
                            nc.sync.dma_start(
                                rot[:, dc, :m],
                                ro_d.ap()[dc * 128:(dc + 1) * 128,
                                          vb * 128:vb * 128 + m])
                        pl0 = pbp.tile([128, TH], F32, tag="pb", name=_nm("pb_"))
                        pl1 = pbp.tile([128, TH], F32, tag="pb", name=_nm("pb_"))
                        pls = [pl0, pl1]
                        for dc in range(ND):
                            for th in range(2):
                                for ns in range(2):
                                    nc.tensor.matmul(
                                        pls[th][:m, ns * 512:(ns + 1) * 512],
                                        rot[:, dc, :m],
                                        vpT[:, dc, th * TH + ns * 512:
                                            th * TH + (ns + 1) * 512],
                                        start=(dc == 0), stop=(dc == ND - 1),
                                        skip_group_check=True)
                        lo = lop.tile([128, T], BF16, tag="lo", name=_nm("lo_"))
                        for th in range(2):
                            nc.scalar.copy(lo[:m, th * TH:(th + 1) * TH], pls[th][:m])
                        nc.sync.dma_start(out_d.ap()[vb * 128:vb * 128 + m, :], lo[:m])

    nc.compile()
    return nc


_NC_CACHE = None


def _get_nc():
    global _NC_CACHE
    if _NC_CACHE is None:
        nc = bacc.Bacc("TRN2", target_bir_lowering=False, debug=False, num_devices=8)
        _NC_CACHE = build(nc)
    return _NC_CACHE


def _rope_tables():
    # match the jax reference: float32 angle computation, then bf16 cast
    import ml_dtypes
    inv_freq = (1.0 / (10000.0 ** (np.arange(0, K, 2, dtype=np.float32)
                                   / np.float32(K)))).astype(np.float32)
    t = np.arange(T, dtype=np.float32)
    freqs = (t[:, None] * inv_freq[None, :]).astype(np.float32)  # [T, K/2]
    cos = np.cos(freqs).astype(np.float32)
    sin = np.sin(freqs).astype(np.float32)
    # [K/2, T] -> [4, 128, 2, 1024] -> [8, 128, 1024] with index i*2+th
    def pack(a):
        aT = np.ascontiguousarray(a.T).reshape(4, 128, 2, TH)
        return np.ascontiguousarray(
            aT.transpose(0, 2, 1, 3).reshape(8, 128, TH)).astype(ml_dtypes.bfloat16)
    return pack(cos), pack(sin)


def kernel(input_, emb, pos, Dx, Dy, E, readout):
    import ml_dtypes
    BF = ml_dtypes.bfloat16
    input_ = np.asarray(input_)
    emb = np.ascontiguousarray(np.asarray(emb, dtype=np.float32))
    pos = np.ascontiguousarray(np.asarray(pos, dtype=np.float32))
    Dx = np.asarray(Dx, dtype=np.float32)
    Dy = np.asarray(Dy, dtype=np.float32)
    E = np.asarray(E, dtype=np.float32)
    readout = np.asarray(readout, dtype=np.float32)

    nc = _get_nc()
    cosb, sinb = _rope_tables()
    ro_bf = readout.astype(BF)

    in_maps = []
    for c in range(8):
        b, h = divmod(c, 4)
        in_maps.append({
            "tok": np.ascontiguousarray(input_[b].astype(np.int32)),
            "emb": emb,
            "pos": pos,
            "dxb": np.ascontiguousarray(Dx[h].astype(BF)),
            "dyb": np.ascontiguousarray(Dy[h].astype(BF)),
            "eb": np.ascontiguousarray(E[h * K:(h + 1) * K].astype(BF)),
            "rob": np.ascontiguousarray(ro_bf[:, h * VS:(h + 1) * VS]),
            "cosb": cosb,
            "sinb": sinb,
        })
    trace = os.environ.get("KRN_TRACE", "0") == "1"
    res = run_bass_kernel_spmd(nc, in_maps, list(range(8)), trace=trace)
    out = np.empty((B, T, V), dtype=np.float32)
    for c in range(8):
        b, h = divmod(c, 4)
        out[b, :, h * VS:(h + 1) * VS] = res.results[c]["logitsT"].astype(np.float32).T
    kernel._last_results = res
    return out


# revision 9
# speedup vs baseline: 1.5543x; 1.0047x over previous
"""Trainium2 Bass kernel for nn_BDH_4406636445711 (dense transformer).

Sharding: 8 cores = data-parallel over B(2) x tensor-parallel over H(4).
Core c handles (b = c//4, h = c%4): its head's Dx/Dy slices, E rows, and a
V/4 shard of the readout. Per layer the y@E partial is AllReduced (bf16)
within each b-group of 4 cores. The host stitches the 8 per-core [VS, T]
logit shards (bf16 on device, cast to fp32 host-side) into [B, T, V].

Key algebraic optimization vs the naive graph: scores = q @ q^T is only
ever used for a = scores @ v, so we compute a = q @ (q^T v) instead --
G = q^T v is [K, D]; ~5x fewer PE cycles than materializing [T, T] scores.

Layouts: v lives token-major ("td": [128 tok part, 16 chunk, 256 d]) so
every LayerNorm is a free-dim bn_stats reduction. x/q live kT; q is also
transposed to tk for the G matmul. All matmul operands are bf16 (full PE
rate); accumulation and v/LN stats stay fp32. ACT engine only uses
{Relu, Copy, Sqrt, Identity} so its function table never reloads.

Pipeline: the layer is software-pipelined around the two AllReduces --
a[8..15] + D(th1) run during cc0's flight; E(th0)/B(th0) and the first
half of the NEXT layer's G run during cc1's flight.
"""

import os
import sys

sys.path.insert(0, "/opt/trn_rl_repo")

import numpy as np

import concourse.bass as bass
import concourse.tile as tile
from concourse import bacc, mybir
from concourse.bass_utils import run_bass_kernel_spmd
from concourse.masks import make_identity
from concourse import library_config

F32 = mybir.dt.float32
BF16 = mybir.dt.bfloat16
I32 = mybir.dt.int32
AF = mybir.ActivationFunctionType
OP = mybir.AluOpType

B, T, H, D, K, V, L = 2, 2048, 4, 256, 1024, 32000, 6
VS = V // 4          # vocab shard per core within a b-group
EPS = 1e-5
NT = T // 128        # 16 token chunks
NKC = K // 128       # 8 k chunks
ND = D // 128        # 2 d chunks
TH = T // 2          # 1024

N_LAYERS = int(os.environ.get("KRN_LAYERS", str(L)))
DO_READOUT = os.environ.get("KRN_READOUT", "1") == "1"


def build(nc):
    # ---- DRAM parameters (per core) ----
    tok_d = nc.dram_tensor("tok", [T], I32, kind="ExternalInput")
    emb_d = nc.dram_tensor("emb", [V, D], F32, kind="ExternalInput")
    pos_d = nc.dram_tensor("pos", [T, D], F32, kind="ExternalInput")
    dx_d = nc.dram_tensor("dxb", [D, K], BF16, kind="ExternalInput")
    dy_d = nc.dram_tensor("dyb", [D, K], BF16, kind="ExternalInput")
    e_d = nc.dram_tensor("eb", [K, D], BF16, kind="ExternalInput")
    ro_d = nc.dram_tensor("rob", [D, VS], BF16, kind="ExternalInput")
    cos_d = nc.dram_tensor("cosb", [8, 128, TH], BF16, kind="ExternalInput")
    sin_d = nc.dram_tensor("sinb", [8, 128, TH], BF16, kind="ExternalInput")
    out_d = nc.dram_tensor("logitsT", [VS, T], BF16, kind="ExternalOutput")

    groups = [[0, 1, 2, 3], [4, 5, 6, 7]]

    with tile.TileContext(nc) as tc:
        with (
            nc.allow_low_precision(reason="bf16 matmul path is intentional"),
            tc.tile_pool(name="persist", bufs=1) as pp,
            tc.tile_pool(name="w4", bufs=6) as w4p,     # [128,1024] bf16 rope
            tc.tile_pool(name="sm", bufs=6) as smp,     # [128,256] f32 temps
            tc.tile_pool(name="lnt", bufs=3) as lntp,   # [128,256] bf16 lnA_td
            tc.tile_pool(name="stats", bufs=12) as stp, # [128,8] f32
            tc.tile_pool(name="rop", bufs=3) as rop,    # readout weights
            tc.tile_pool(name="lop", bufs=2) as lop,    # logit staging
            tc.tile_pool(name="pb", bufs=2, space="PSUM") as pbp,   # [128,1024] f32
            tc.tile_pool(name="pa", bufs=2, space="PSUM") as pap,   # [128,256] f32
            tc.tile_pool(name="pt", bufs=2, space="PSUM") as ptp,   # [128,8,128] bf16
            tc.tile_pool(name="dram", bufs=1, space="DRAM") as dpool,
        ):
            _ctr = [0]

            def _nm(p):
                _ctr[0] += 1
                return f"{p}{_ctr[0]}"

            # ---- constants ----
            ident_f = smp.tile([128, 128], F32, tag="sm", name="identf")
            make_identity(nc, ident_f[:])
            ident_bf = pp.tile([128, 128], BF16)
            nc.vector.tensor_copy(ident_bf[:], ident_f[:])
            eps_p = pp.tile([128, 1], F32)
            nc.vector.memset(eps_p[:], EPS)
            nc.gpsimd.load_library(library_config.attn)

            # ---- persistent tensors ----
            v_td = pp.tile([128, NT, D], F32)          # v (+pos), token-major
            vp_bf = pp.tile([128, NT, D], BF16)        # bf16 cast of v_td
            vpT = pp.tile([128, ND, T], BF16)          # v transposed (d-major)
            qT = pp.tile([128, NKC, T], BF16)          # q k-major; reused as yt
            qtk = pp.tile([128, NT, NKC, 128], BF16)   # q token-major
            x_bf = pp.tile([128, NKC, T], BF16)        # relu(v@Dx), k-major
            g0_sb = pp.tile([128, NKC, D], F32)        # G partial (th0 tokens)
            g_bf = pp.tile([128, NKC, D], BF16)        # G = q^T (v+pos), full
            lnA_dT = pp.tile([128, ND, T], BF16)       # ln(a) d-major
            pos_sb = pp.tile([128, NT, D], F32)
            dx_sb = pp.tile([128, ND, K], BF16)
            dy_sb = pp.tile([128, ND, K], BF16)
            e_sb = pp.tile([128, NKC, D], BF16)

            nc.sync.dma_start(pos_sb[:], pos_d.ap().rearrange("(j p) d -> p j d", p=128))
            nc.sync.dma_start(dx_sb[:], dx_d.ap().rearrange("(c p) k -> p c k", p=128))
            nc.sync.dma_start(dy_sb[:], dy_d.ap().rearrange("(c p) k -> p c k", p=128))
            nc.sync.dma_start(e_sb[:], e_d.ap().rearrange("(c p) d -> p c d", p=128))

            # ---- internal DRAM (collective staging, bf16) ----
            cc_in = [dpool.tile([8, 128, D], BF16, tag=f"cci{i}", name=f"cci{i}")
                     for i in range(2)]
            cc_out = [dpool.tile([8, 128, D], BF16, tag=f"cco{i}", name=f"cco{i}")
                      for i in range(2)]

            def ln_stats(src_ap):
                """rstd and -mean*rstd ([128,1] f32) of a [128, D] td chunk."""
                st = stp.tile([128, 8], F32, tag="st", name=_nm("st_"))
                nc.vector.bn_stats(st[:, 0:6], src_ap)
                mv = stp.tile([128, 8], F32, tag="st", name=_nm("st_"))
                nc.vector.bn_aggr(mv[:, 0:2], st[:, 0:6])
                sd = stp.tile([128, 8], F32, tag="st", name=_nm("st_"))
                nc.scalar.activation(sd[:, 0:1], mv[:, 1:2], AF.Sqrt, bias=eps_p[:])
                rstd = stp.tile([128, 8], F32, tag="st", name=_nm("st_"))
                nc.vector.reciprocal(rstd[:, 0:1], sd[:, 0:1])
                nmr = stp.tile([128, 8], F32, tag="st", name=_nm("st_"))
                nc.vector.tensor_scalar(
                    out=nmr[:, 0:1], in0=mv[:, 0:1], scalar1=rstd[:, 0:1],
                    scalar2=-1.0, op0=OP.mult, op1=OP.mult)
                return rstd, nmr

            def apply_ln(dst_ap, src_ap, rstd, nmr):
                # dst = (src - mean) * rstd, on the ACT engine
                nc.scalar.activation(dst_ap, src_ap, AF.Identity,
                                     bias=nmr[:, 0:1], scale=rstd[:, 0:1])

            def vchunk_finish(j, src_ap, add_pos):
                """v_td[:,j] = ln(src) (+pos); refresh vp_bf and vpT."""
                rstd, nmr = ln_stats(src_ap)
                if add_pos:
                    tmp = smp.tile([128, D], F32, tag="sm", name=_nm("sm_"))
                    apply_ln(tmp[:], src_ap, rstd, nmr)
                    nc.vector.tensor_add(v_td[:, j], tmp[:], pos_sb[:, j])
                else:
                    apply_ln(v_td[:, j], src_ap, rstd, nmr)
                nc.scalar.copy(vp_bf[:, j], v_td[:, j])
                tp = ptp.tile([128, 8, 128], BF16, tag="pt", name=_nm("pt_"))
                for dc in range(ND):
                    nc.tensor.transpose(tp[:, dc], vp_bf[:, j, dc * 128:(dc + 1) * 128],
                                        ident_bf[:])
                nc.scalar.copy(vpT[:, :, j * 128:(j + 1) * 128], tp[:, 0:2])

            # ================= embedding gather + LN + pos =================
            idx = pp.tile([128, NT], I32)
            nc.sync.dma_start(idx[:], tok_d.ap().rearrange("(n p) -> p n", p=128))
            for j in range(NT):
                gat = smp.tile([128, D], F32, tag="sm", name=_nm("sm_"))
                nc.gpsimd.indirect_dma_start(
                    out=gat[:], out_offset=None, in_=emb_d.ap(),
                    in_offset=bass.IndirectOffsetOnAxis(ap=idx[:, j:j + 1], axis=0),
                )
                vchunk_finish(j, gat[:], add_pos=True)

            def phaseB(th):
                """x[:, th cols] = relu(vp @ Dx); RoPE -> q; build qtk."""
                c0 = th * TH
                for i in range(NKC):
                    px = pbp.tile([128, TH], F32, tag="pb", name=_nm("pb_"))
                    for dc in range(ND):
                        for ns in range(2):
                            nc.tensor.matmul(
                                px[:, ns * 512:(ns + 1) * 512],
                                dx_sb[:, dc, i * 128:(i + 1) * 128],
                                vpT[:, dc, c0 + ns * 512:c0 + (ns + 1) * 512],
                                start=(dc == 0), stop=(dc == ND - 1))
                    nc.scalar.activation(x_bf[:, i, c0:c0 + TH], px[:], AF.Relu)
                for i in range(4):
                    cos_t = w4p.tile([128, TH], BF16, tag="w4", name=_nm("w4_"))
                    nc.sync.dma_start(cos_t[:], cos_d.ap()[i * 2 + th])
                    sin_t = w4p.tile([128, TH], BF16, tag="w4", name=_nm("w4_"))
                    nc.sync.dma_start(sin_t[:], sin_d.ap()[i * 2 + th])
                    xi = x_bf[:, i, c0:c0 + TH]
                    xj = x_bf[:, i + 4, c0:c0 + TH]
                    ma = w4p.tile([128, TH], BF16, tag="w4", name=_nm("w4_"))
                    nc.vector.tensor_mul(ma[:], xi, cos_t[:])
                    mb = w4p.tile([128, TH], BF16, tag="w4", name=_nm("w4_"))
                    nc.vector.tensor_mul(mb[:], xj, sin_t[:])
                    nc.vector.tensor_sub(qT[:, i, c0:c0 + TH], ma[:], mb[:])
                    nc.vector.tensor_mul(ma[:], xj, cos_t[:])
                    nc.vector.tensor_mul(mb[:], xi, sin_t[:])
                    nc.vector.tensor_add(qT[:, i + 4, c0:c0 + TH], ma[:], mb[:])
                for j in range(th * 8, th * 8 + 8):
                    tq = ptp.tile([128, 8, 128], BF16, tag="pt", name=_nm("pt_"))
                    for kc in range(NKC):
                        nc.tensor.transpose(tq[:, kc], qT[:, kc, j * 128:(j + 1) * 128],
                                            ident_bf[:])
                    nc.scalar.copy(qtk[:, j], tq[:])

            def phaseG(half):
                """G half-accumulation over token chunks; half 1 finalizes g_bf."""
                for kc in range(NKC):
                    pg = pap.tile([128, D], F32, tag="pa", name=_nm("pa_"))
                    for j in range(half * 8, half * 8 + 8):
                        nc.tensor.matmul(pg[:], qtk[:, j, kc], vp_bf[:, j],
                                         start=(j == half * 8), stop=(j == half * 8 + 7))
                    if half == 0:
                        nc.scalar.copy(g0_sb[:, kc], pg[:])
                    else:
                        nc.vector.tensor_add(g_bf[:, kc], g0_sb[:, kc], pg[:])

            def phaseCa(half):
                """a = q G (token-major) with fused LN -> lnA_dT, token half."""
                for j in range(half * 8, half * 8 + 8):
                    paa = pap.tile([128, D], F32, tag="pa", name=_nm("pa_"))
                    for kc in range(NKC):
                        nc.tensor.matmul(paa[:], qT[:, kc, j * 128:(j + 1) * 128],
                                         g_bf[:, kc],
                                         start=(kc == 0), stop=(kc == NKC - 1))
                    rstd, nmr = ln_stats(paa[:])
                    lnt = lntp.tile([128, D], BF16, tag="lnt", name=_nm("lnt_"))
                    apply_ln(lnt[:], paa[:], rstd, nmr)
                    tp = ptp.tile([128, 8, 128], BF16, tag="pt", name=_nm("pt_"))
                    for dc in range(ND):
                        nc.tensor.transpose(tp[:, dc], lnt[:, dc * 128:(dc + 1) * 128],
                                            ident_bf[:])
                    nc.scalar.copy(lnA_dT[:, :, j * 128:(j + 1) * 128], tp[:, 0:2])

            def phaseDy(th):
                """y = relu(lnA@Dy)*x into yt (aliases q's buffer)."""
                c0 = th * TH
                yt = qT
                for i in range(NKC):
                    py = pbp.tile([128, TH], F32, tag="pb", name=_nm("pb_"))
                    for dc in range(ND):
                        for ns in range(2):
                            nc.tensor.matmul(
                                py[:, ns * 512:(ns + 1) * 512],
                                dy_sb[:, dc, i * 128:(i + 1) * 128],
                                lnA_dT[:, dc, c0 + ns * 512:c0 + (ns + 1) * 512],
                                start=(dc == 0), stop=(dc == ND - 1))
                    nc.vector.scalar_tensor_tensor(
                        out=yt[:, i, c0:c0 + TH], in0=py[:], scalar=0.0,
                        in1=x_bf[:, i, c0:c0 + TH], op0=OP.max, op1=OP.mult)

            def phaseDu(th):
                """u = y@E (token-major); stage bf16 and AllReduce."""
                yt = qT
                for j in range(th * 8, th * 8 + 8):
                    pu = pap.tile([128, D], F32, tag="pa", name=_nm("pa_"))
                    for i in range(NKC):
                        nc.tensor.matmul(pu[:], yt[:, i, j * 128:(j + 1) * 128],
                                         e_sb[:, i],
                                         start=(i == 0), stop=(i == NKC - 1))
                    us = smp.tile([128, D], BF16, tag="smb", bufs=4,
                                  name=_nm("smb_"))
                    nc.scalar.copy(us[:], pu[:])
                    nc.sync.dma_start(cc_in[th][:][j - th * 8], us[:])
                nc.gpsimd.collective_compute(
                    "AllReduce", OP.add, replica_groups=groups,
                    ins=[cc_in[th][:].opt()], outs=[cc_out[th][:].opt()])

            def phaseE(th, layer):
                """v = ln(v + ln(allreduce(u))) (+pos); refresh vp/vpT."""
                last = layer == N_LAYERS - 1
                for j in range(th * 8, th * 8 + 8):
                    us = smp.tile([128, D], BF16, tag="smb", bufs=4,
                                  name=_nm("smb_"))
                    nc.sync.dma_start(us[:], cc_out[th][:][j - th * 8])
                    rstd, nmr = ln_stats(us[:])
                    uln = smp.tile([128, D], F32, tag="sm", name=_nm("sm_"))
                    apply_ln(uln[:], us[:], rstd, nmr)
                    nc.vector.tensor_add(v_td[:, j], v_td[:, j], uln[:])
                    vchunk_finish(j, v_td[:, j], add_pos=not last)

            # ================================ layers ================================
            phaseB(0)
            phaseB(1)
            phaseG(0)
            phaseG(1)
            for layer in range(N_LAYERS):
                last = layer == N_LAYERS - 1
                with nc.named_scope(f"L{layer}"):
                    phaseCa(0)
                    phaseDy(0)
                    phaseDu(0)      # cc0 in flight...
                    phaseCa(1)      # ...covered by a[8..15] + D(th1)
                    phaseDy(1)
                    phaseDu(1)      # cc1 in flight...
                    phaseE(0, layer)
                    if not last:
                        phaseB(0)   # ...covered by E/B(th0) + next G half
                        phaseG(0)
                    phaseE(1, layer)
                    if not last:
                        phaseB(1)
                        phaseG(1)

            # ============= readout: logitsT = (v @ readout)^T, V-sharded =============
            if DO_READOUT:
                nvb = (VS + 127) // 128
                with nc.named_scope("readout"):
                    for vb in range(nvb):
                        m = min(128, VS - vb * 128)
                        rot = rop.tile([128, ND, 128], BF16, tag="ro", name=_nm("ro_"))
                        for dc in range(ND):
                            nc.sync.dma_start(
                                rot[:, dc, :m],
                                ro_d.ap()[dc * 128:(dc + 1) * 128,
                                          vb * 128:vb * 128 + m])
                        pl0 = pbp.tile([128, TH], F32, tag="pb", name=_nm("pb_"))
                        pl1 = pbp.tile([128, TH], F32, tag="pb", name=_nm("pb_"))
                        pls = [pl0, pl1]
                        for dc in range(ND):
                            for th in range(2):
                                for ns in range(2):
                                    nc.tensor.matmul(
                                        pls[th][:m, ns * 512:(ns + 1) * 512],
                                        rot[:, dc, :m],
                                        vpT[:, dc, th * TH + ns * 512:
                                            th * TH + (ns + 1) * 512],
                                        start=(dc == 0), stop=(dc == ND - 1),
                                        skip_group_check=True)
                        lo = lop.tile([128, T], BF16, tag="lo", name=_nm("lo_"))
                        nc.scalar.copy(lo[:m, 0:TH], pl0[:m])
                        nc.vector.tensor_copy(lo[:m, TH:T], pl1[:m])
                        nc.sync.dma_start(out_d.ap()[vb * 128:vb * 128 + m, :], lo[:m])

    nc.compile()
    return nc


_NC_CACHE = None


def _get_nc():
    global _NC_CACHE
    if _NC_CACHE is None:
        nc = bacc.Bacc("TRN2", target_bir_lowering=False, debug=False, num_devices=8)
        _NC_CACHE = build(nc)
    return _NC_CACHE


def _rope_tables():
    # match the jax reference: float32 angle computation, then bf16 cast
    import ml_dtypes
    inv_freq = (1.0 / (10000.0 ** (np.arange(0, K, 2, dtype=np.float32)
                                   / np.float32(K)))).astype(np.float32)
    t = np.arange(T, dtype=np.float32)
    freqs = (t[:, None] * inv_freq[None, :]).astype(np.float32)  # [T, K/2]
    cos = np.cos(freqs).astype(np.float32)
    sin = np.sin(freqs).astype(np.float32)
    # [K/2, T] -> [4, 128, 2, 1024] -> [8, 128, 1024] with index i*2+th
    def pack(a):
        aT = np.ascontiguousarray(a.T).reshape(4, 128, 2, TH)
        return np.ascontiguousarray(
            aT.transpose(0, 2, 1, 3).reshape(8, 128, TH)).astype(ml_dtypes.bfloat16)
    return pack(cos), pack(sin)


def kernel(input_, emb, pos, Dx, Dy, E, readout):
    import ml_dtypes
    BF = ml_dtypes.bfloat16
    input_ = np.asarray(input_)
    emb = np.ascontiguousarray(np.asarray(emb, dtype=np.float32))
    pos = np.ascontiguousarray(np.asarray(pos, dtype=np.float32))
    Dx = np.asarray(Dx, dtype=np.float32)
    Dy = np.asarray(Dy, dtype=np.float32)
    E = np.asarray(E, dtype=np.float32)
    readout = np.asarray(readout, dtype=np.float32)

    nc = _get_nc()
    cosb, sinb = _rope_tables()
    ro_bf = readout.astype(BF)

    in_maps = []
    for c in range(8):
        b, h = divmod(c, 4)
        in_maps.append({
            "tok": np.ascontiguousarray(input_[b].astype(np.int32)),
            "emb": emb,
            "pos": pos,
            "dxb": np.ascontiguousarray(Dx[h].astype(BF)),
            "dyb": np.ascontiguousarray(Dy[h].astype(BF)),
            "eb": np.ascontiguousarray(E[h * K:(h + 1) * K].astype(BF)),
            "rob": np.ascontiguousarray(ro_bf[:, h * VS:(h + 1) * VS]),
            "cosb": cosb,
            "sinb": sinb,
        })
    trace = os.environ.get("KRN_TRACE", "0") == "1"
    res = run_bass_kernel_spmd(nc, in_maps, list(range(8)), trace=trace)
    out = np.empty((B, T, V), dtype=np.float32)
    for c in range(8):
        b, h = divmod(c, 4)
        out[b, :, h * VS:(h + 1) * VS] = res.results[c]["logitsT"].astype(np.float32).T
    kernel._last_results = res
    return out


# revision 12
# speedup vs baseline: 1.6405x; 1.0555x over previous
"""Trainium2 Bass kernel for nn_BDH_4406636445711 (dense transformer).

Sharding: 8 cores = data-parallel over B(2) x tensor-parallel over H(4).
Core c handles (b = c//4, h = c%4): its head's Dx/Dy slices, E rows, and a
V/4 shard of the readout. Per layer the y@E partial is AllReduced (bf16)
within each b-group of 4 cores. The host stitches the 8 per-core [VS, T]
logit shards (bf16 on device, cast to fp32 host-side) into [B, T, V].

Key algebraic optimization vs the naive graph: scores = q @ q^T is only
ever used for a = scores @ v, so we compute a = q @ (q^T v) instead --
G = q^T v is [K, D]; ~5x fewer PE cycles than materializing [T, T] scores.

Layouts: v lives token-major ("td": [128 tok part, 16 chunk, 256 d]) so
every LayerNorm is a free-dim reduction. x/q live kT; q is additionally
transposed to tk for the G matmul. All matmul operands are bf16 (full PE
rate); accumulation and LN stats stay fp32. ACT only ever needs the
{relu, copy, sqrt, square, identity} table -- no table reloads.

All LN statistics are batched per 8-chunk half (sums via one DVE
tensor_reduce or free ACT accum_out on copies; sum-of-squares via ACT
Square+accum_out; the scalar chain runs once per half on [128,8] tiles)
to avoid per-chunk cross-engine ping-pong on the in-order queues.
The layer is software-pipelined around the two AllReduces.
"""

import os
import sys

sys.path.insert(0, "/opt/trn_rl_repo")

import numpy as np

import concourse.bass as bass
import concourse.tile as tile
from concourse import bacc, mybir
from concourse.bass_utils import run_bass_kernel_spmd
from concourse.masks import make_identity
from concourse import library_config

F32 = mybir.dt.float32
BF16 = mybir.dt.bfloat16
I32 = mybir.dt.int32
AF = mybir.ActivationFunctionType
OP = mybir.AluOpType
AX = mybir.AxisListType

B, T, H, D, K, V, L = 2, 2048, 4, 256, 1024, 32000, 6
VS = V // 4          # vocab shard per core within a b-group
EPS = 1e-5
NT = T // 128        # 16 token chunks
NKC = K // 128       # 8 k chunks
ND = D // 128        # 2 d chunks
TH = T // 2          # 1024

N_LAYERS = int(os.environ.get("KRN_LAYERS", str(L)))
DO_READOUT = os.environ.get("KRN_READOUT", "1") == "1"


def build(nc):
    # ---- DRAM parameters (per core) ----
    tok_d = nc.dram_tensor("tok", [T], I32, kind="ExternalInput")
    emb_d = nc.dram_tensor("emb", [V, D], F32, kind="ExternalInput")
    pos_d = nc.dram_tensor("posb", [T, D], BF16, kind="ExternalInput")
    dx_d = nc.dram_tensor("dxb", [D, K], BF16, kind="ExternalInput")
    dy_d = nc.dram_tensor("dyb", [D, K], BF16, kind="ExternalInput")
    e_d = nc.dram_tensor("eb", [K, D], BF16, kind="ExternalInput")
    ro_d = nc.dram_tensor("rob", [D, VS], BF16, kind="ExternalInput")
    cos_d = nc.dram_tensor("cosb", [8, 128, TH], BF16, kind="ExternalInput")
    sin_d = nc.dram_tensor("sinb", [8, 128, TH], BF16, kind="ExternalInput")
    out_d = nc.dram_tensor("logitsT", [VS, T], BF16, kind="ExternalOutput")

    groups = [[0, 1, 2, 3], [4, 5, 6, 7]]

    with tile.TileContext(nc) as tc:
        with (
            nc.allow_low_precision(reason="bf16 matmul path is intentional"),
            tc.tile_pool(name="persist", bufs=1) as pp,
            tc.tile_pool(name="w4", bufs=4) as w4p,     # [128,1024] bf16 rope
            tc.tile_pool(name="sm", bufs=4) as smp,     # [128,256] f32 gathers
            tc.tile_pool(name="stats", bufs=16) as stp, # [128,8] f32
            tc.tile_pool(name="rop", bufs=2) as rop,    # readout weights
            tc.tile_pool(name="lop", bufs=2) as lop,    # logit staging
            tc.tile_pool(name="pb", bufs=2, space="PSUM") as pbp,   # [128,1024] f32
            tc.tile_pool(name="pa", bufs=2, space="PSUM") as pap,   # [128,256] f32
            tc.tile_pool(name="pt", bufs=2, space="PSUM") as ptp,   # [128,1024] bf16
            tc.tile_pool(name="dram", bufs=1, space="DRAM") as dpool,
        ):
            _ctr = [0]

            def _nm(p):
                _ctr[0] += 1
                return f"{p}{_ctr[0]}"

            # ---- constants ----
            ident_f = smp.tile([128, 128], F32, tag="sm", name="identf")
            make_identity(nc, ident_f[:])
            ident_bf = pp.tile([128, 128], BF16)
            nc.vector.tensor_copy(ident_bf[:], ident_f[:])
            eps_p = pp.tile([128, 1], F32)
            nc.vector.memset(eps_p[:], EPS)
            nc.gpsimd.load_library(library_config.attn)

            # ---- persistent tensors ----
            v_td = pp.tile([128, NT, D], F32)          # v (+pos), token-major
            vp_bf = pp.tile([128, NT, D], BF16)        # bf16 cast of v_td
            vpT = pp.tile([128, ND, T], BF16)          # v transposed (d-major)
            qT = pp.tile([128, NKC, T], BF16)          # q k-major; reused as yt
            qtk = pp.tile([128, NT, K], BF16)          # q token-major
            x_bf = pp.tile([128, NKC, T], BF16)        # relu(v@Dx), k-major
            g0_sb = pp.tile([128, NKC, D], BF16)       # G partial (th0 tokens)
            g_bf = pp.tile([128, NKC, D], BF16)        # G = q^T (v+pos), full
            lnA_dT = pp.tile([128, ND, T], BF16)       # ln(a) d-major
            pos_sb = pp.tile([128, NT, D], BF16)
            dx_sb = pp.tile([128, ND, K], BF16)
            dy_sb = pp.tile([128, ND, K], BF16)
            e_sb = pp.tile([128, NKC, D], BF16)
            # half-batch scratch ([128, 8, 256] = one token half)
            a_all = pp.tile([128, 8, D], BF16)         # a copies / u staging
            u_all = pp.tile([128, 8, D], BF16)         # allreduced u
            uln_all = pp.tile([128, 8, D], F32)        # ln(u)
            lnt_all = pp.tile([128, 8, D], BF16)       # ln(a) token-major
            junk = pp.tile([128, D], BF16)             # ACT Square sink

            nc.sync.dma_start(pos_sb[:], pos_d.ap().rearrange("(j p) d -> p j d", p=128))
            nc.sync.dma_start(dx_sb[:], dx_d.ap().rearrange("(c p) k -> p c k", p=128))
            nc.sync.dma_start(dy_sb[:], dy_d.ap().rearrange("(c p) k -> p c k", p=128))
            nc.sync.dma_start(e_sb[:], e_d.ap().rearrange("(c p) d -> p c d", p=128))

            # ---- internal DRAM (collective staging, bf16) ----
            cc_in = [dpool.tile([8, 128, D], BF16, tag=f"cci{i}", name=f"cci{i}")
                     for i in range(2)]
            cc_out = [dpool.tile([8, 128, D], BF16, tag=f"cco{i}", name=f"cco{i}")
                      for i in range(2)]

            def stats_batch(sum_t, sq_t, n=8):
                """Batched LN stats: rstd, -mean*rstd as [128, n] f32 tiles."""
                negm = stp.tile([128, 8], F32, tag="st", name=_nm("st_"))
                nc.vector.tensor_scalar_mul(negm[:, :n], sum_t, -1.0 / D)
                msq = stp.tile([128, 8], F32, tag="st", name=_nm("st_"))
                nc.vector.tensor_mul(msq[:, :n], negm[:, :n], negm[:, :n])
                var = stp.tile([128, 8], F32, tag="st", name=_nm("st_"))
                nc.vector.scalar_tensor_tensor(
                    out=var[:, :n], in0=sq_t, scalar=1.0 / D, in1=msq[:, :n],
                    op0=OP.mult, op1=OP.subtract)
                sd = stp.tile([128, 8], F32, tag="st", name=_nm("st_"))
                nc.scalar.activation(sd[:, :n], var[:, :n], AF.Sqrt, bias=eps_p[:])
                rstd = stp.tile([128, 8], F32, tag="st", name=_nm("st_"))
                nc.vector.reciprocal(rstd[:, :n], sd[:, :n])
                nmr = stp.tile([128, 8], F32, tag="st", name=_nm("st_"))
                nc.vector.tensor_mul(nmr[:, :n], negm[:, :n], rstd[:, :n])
                return rstd, nmr

            def sq_accum(src_ap, sq_t, jj):
                """sum(src^2) -> sq_t[:, jj] via ACT Square + accumulator."""
                nc.scalar.activation(junk[:], src_ap, AF.Square,
                                     accum_out=sq_t[:, jj:jj + 1])

            def transpose_half(src_tile, sl, dst, c0):
                """Transpose 8 [128, 256] td chunks into dst[:, dc, c0:c0+1024]."""
                tpa = ptp.tile([128, TH], BF16, tag="pt", name=_nm("pt_"))
                tpb = ptp.tile([128, TH], BF16, tag="pt", name=_nm("pt_"))
                for j8 in range(8):
                    nc.tensor.transpose(tpa[:, j8 * 128:(j8 + 1) * 128],
                                        src_tile[:, sl + j8, 0:128], ident_bf[:])
                    nc.tensor.transpose(tpb[:, j8 * 128:(j8 + 1) * 128],
                                        src_tile[:, sl + j8, 128:256], ident_bf[:])
                nc.scalar.copy(dst[:, 0, c0:c0 + TH], tpa[:])
                nc.scalar.copy(dst[:, 1, c0:c0 + TH], tpb[:])

            # ================= embedding gather + LN + pos =================
            idx = pp.tile([128, NT], I32)
            nc.sync.dma_start(idx[:], tok_d.ap().rearrange("(n p) -> p n", p=128))
            for th in range(2):
                h0 = th * 8
                sum_t = stp.tile([128, 8], F32, tag="st", name=_nm("st_"))
                sq_t = stp.tile([128, 8], F32, tag="st", name=_nm("st_"))
                for j8 in range(8):
                    nc.gpsimd.indirect_dma_start(
                        out=uln_all[:, j8], out_offset=None, in_=emb_d.ap(),
                        in_offset=bass.IndirectOffsetOnAxis(
                            ap=idx[:, h0 + j8:h0 + j8 + 1], axis=0),
                    )
                    nc.scalar.activation(junk[:], uln_all[:, j8], AF.Copy,
                                         accum_out=sum_t[:, j8:j8 + 1])
                    sq_accum(uln_all[:, j8], sq_t, j8)
                rstd, nmr = stats_batch(sum_t[:], sq_t[:])
                for j8 in range(8):
                    nc.scalar.activation(v_td[:, h0 + j8], uln_all[:, j8],
                                         AF.Identity, bias=nmr[:, j8:j8 + 1],
                                         scale=rstd[:, j8:j8 + 1])
                sl = slice(h0, h0 + 8)
                nc.vector.tensor_add(v_td[:, sl], v_td[:, sl], pos_sb[:, sl])
                nc.scalar.copy(vp_bf[:, sl], v_td[:, sl])
                transpose_half(vp_bf, h0, vpT, th * TH)

            def phaseB(th):
                """x[:, th cols] = relu(vp @ Dx); RoPE -> q; build qtk."""
                c0 = th * TH

                def px_one(i):
                    px = pbp.tile([128, TH], F32, tag="pb", name=_nm("pb_"))
                    for dc in range(ND):
                        for ns in range(2):
                            nc.tensor.matmul(
                                px[:, ns * 512:(ns + 1) * 512],
                                dx_sb[:, dc, i * 128:(i + 1) * 128],
                                vpT[:, dc, c0 + ns * 512:c0 + (ns + 1) * 512],
                                start=(dc == 0), stop=(dc == ND - 1))
                    nc.scalar.activation(x_bf[:, i, c0:c0 + TH], px[:], AF.Relu)

                def rope_one(i):
                    cos_t = w4p.tile([128, TH], BF16, tag="w4", name=_nm("w4_"))
                    nc.sync.dma_start(cos_t[:], cos_d.ap()[i * 2 + th])
                    sin_t = w4p.tile([128, TH], BF16, tag="w4", name=_nm("w4_"))
                    nc.sync.dma_start(sin_t[:], sin_d.ap()[i * 2 + th])
                    xi = x_bf[:, i, c0:c0 + TH]
                    xj = x_bf[:, i + 4, c0:c0 + TH]
                    ma = w4p.tile([128, TH], BF16, tag="w4", name=_nm("w4_"))
                    nc.vector.tensor_mul(ma[:], xi, cos_t[:])
                    mb = w4p.tile([128, TH], BF16, tag="w4", name=_nm("w4_"))
                    nc.vector.tensor_mul(mb[:], xj, sin_t[:])
                    nc.vector.tensor_sub(qT[:, i, c0:c0 + TH], ma[:], mb[:])
                    nc.vector.tensor_mul(ma[:], xj, cos_t[:])
                    nc.vector.tensor_mul(mb[:], xi, sin_t[:])
                    nc.vector.tensor_add(qT[:, i + 4, c0:c0 + TH], ma[:], mb[:])

                # pair-interleaved so rope(i) can start while px continues
                px_one(0); px_one(4); rope_one(0)
                px_one(1); px_one(5); rope_one(1)
                px_one(2); px_one(6); rope_one(2)
                px_one(3); px_one(7); rope_one(3)
                for j in range(th * 8, th * 8 + 8):
                    tq = ptp.tile([128, K], BF16, tag="pt", name=_nm("pt_"))
                    for kc in range(NKC):
                        nc.tensor.transpose(tq[:, kc * 128:(kc + 1) * 128],
                                            qT[:, kc, j * 128:(j + 1) * 128],
                                            ident_bf[:])
                    nc.vector.tensor_copy(qtk[:, j], tq[:])

            def phaseG(half):
                """G half-accumulation over token chunks; half 1 finalizes g_bf."""
                for kc in range(NKC):
                    pg = pap.tile([128, D], F32, tag="pa", name=_nm("pa_"))
                    for j in range(half * 8, half * 8 + 8):
                        nc.tensor.matmul(pg[:], qtk[:, j, kc * 128:(kc + 1) * 128],
                                         vp_bf[:, j],
                                         start=(j == half * 8), stop=(j == half * 8 + 7))
                    if half == 0:
                        nc.scalar.copy(g0_sb[:, kc], pg[:])
                    else:
                        nc.vector.tensor_add(g_bf[:, kc], g0_sb[:, kc], pg[:])

            def phaseCa(half):
                """a = q G (token-major) with batched LN -> lnA_dT, token half."""
                h0 = half * 8
                sum_t = stp.tile([128, 8], F32, tag="st", name=_nm("st_"))
                sq_t = stp.tile([128, 8], F32, tag="st", name=_nm("st_"))
                for j8 in range(8):
                    j = h0 + j8
                    paa = pap.tile([128, D], F32, tag="pa", name=_nm("pa_"))
                    for kc in range(NKC):
                        nc.tensor.matmul(paa[:], qT[:, kc, j * 128:(j + 1) * 128],
                                         g_bf[:, kc],
                                         start=(kc == 0), stop=(kc == NKC - 1))
                    nc.scalar.activation(a_all[:, j8], paa[:], AF.Copy,
                                         accum_out=sum_t[:, j8:j8 + 1])
                    sq_accum(a_all[:, j8], sq_t, j8)
                rstd, nmr = stats_batch(sum_t[:], sq_t[:])
                for j8 in range(8):
                    nc.scalar.activation(lnt_all[:, j8], a_all[:, j8], AF.Identity,
                                         bias=nmr[:, j8:j8 + 1],
                                         scale=rstd[:, j8:j8 + 1])
                transpose_half(lnt_all, 0, lnA_dT, half * TH)

            def phaseDy(th):
                """y = relu(lnA@Dy)*x into yt (aliases q's buffer)."""
                c0 = th * TH
                yt = qT
                for i in range(NKC):
                    py = pbp.tile([128, TH], F32, tag="pb", name=_nm("pb_"))
                    for dc in range(ND):
                        for ns in range(2):
                            nc.tensor.matmul(
                                py[:, ns * 512:(ns + 1) * 512],
                                dy_sb[:, dc, i * 128:(i + 1) * 128],
                                lnA_dT[:, dc, c0 + ns * 512:c0 + (ns + 1) * 512],
                                start=(dc == 0), stop=(dc == ND - 1))
                    nc.vector.scalar_tensor_tensor(
                        out=yt[:, i, c0:c0 + TH], in0=py[:], scalar=0.0,
                        in1=x_bf[:, i, c0:c0 + TH], op0=OP.max, op1=OP.mult)

            def phaseDu(th):
                """u = y@E (token-major); stage bf16 (a_all) and AllReduce."""
                yt = qT
                for j8 in range(8):
                    j = th * 8 + j8
                    pu = pap.tile([128, D], F32, tag="pa", name=_nm("pa_"))
                    for i in range(NKC):
                        nc.tensor.matmul(pu[:], yt[:, i, j * 128:(j + 1) * 128],
                                         e_sb[:, i],
                                         start=(i == 0), stop=(i == NKC - 1))
                    nc.scalar.copy(a_all[:, j8], pu[:])
                nc.sync.dma_start(
                    cc_in[th][:].rearrange("j p d -> p j d"), a_all[:])
                nc.gpsimd.collective_compute(
                    "AllReduce", OP.add, replica_groups=groups,
                    ins=[cc_in[th][:].opt()], outs=[cc_out[th][:].opt()])

            def phaseE(th, layer):
                """v = ln(v + ln(allreduce(u))) (+pos); refresh vp/vpT."""
                last = layer == N_LAYERS - 1
                h0 = th * 8
                sl = slice(h0, h0 + 8)
                nc.sync.dma_start(u_all[:],
                                  cc_out[th][:].rearrange("j p d -> p j d"))
                sum_t = stp.tile([128, 8], F32, tag="st", name=_nm("st_"))
                nc.vector.tensor_reduce(sum_t[:], u_all[:], axis=AX.X, op=OP.add)
                sq_t = stp.tile([128, 8], F32, tag="st", name=_nm("st_"))
                for j8 in range(8):
                    sq_accum(u_all[:, j8], sq_t, j8)
                rstd, nmr = stats_batch(sum_t[:], sq_t[:])
                for j8 in range(8):
                    nc.scalar.activation(uln_all[:, j8], u_all[:, j8], AF.Identity,
                                         bias=nmr[:, j8:j8 + 1],
                                         scale=rstd[:, j8:j8 + 1])
                nc.vector.tensor_add(v_td[:, sl], v_td[:, sl], uln_all[:])
                sum_w = stp.tile([128, 8], F32, tag="st", name=_nm("st_"))
                nc.vector.tensor_reduce(sum_w[:], v_td[:, sl], axis=AX.X, op=OP.add)
                sq_w = stp.tile([128, 8], F32, tag="st", name=_nm("st_"))
                for j8 in range(8):
                    sq_accum(v_td[:, h0 + j8], sq_w, j8)
                rstd_w, nmr_w = stats_batch(sum_w[:], sq_w[:])
                for j8 in range(8):
                    nc.scalar.activation(v_td[:, h0 + j8], v_td[:, h0 + j8],
                                         AF.Identity, bias=nmr_w[:, j8:j8 + 1],
                                         scale=rstd_w[:, j8:j8 + 1])
                if not last:
                    nc.vector.tensor_add(v_td[:, sl], v_td[:, sl], pos_sb[:, sl])
                nc.scalar.copy(vp_bf[:, sl], v_td[:, sl])
                transpose_half(vp_bf, h0, vpT, th * TH)

            # ================================ layers ================================
            phaseB(0)
            phaseB(1)
            phaseG(0)
            phaseG(1)
            for layer in range(N_LAYERS):
                last = layer == N_LAYERS - 1
                with nc.named_scope(f"L{layer}"):
                    phaseCa(0)
                    phaseDy(0)
                    phaseDu(0)      # cc0 in flight...
                    phaseCa(1)      # ...covered by a[8..15] + D(th1)
                    phaseDy(1)
                    phaseDu(1)      # cc1 in flight...
                    phaseE(0, layer)
                    if not last:
                        phaseB(0)   # ...covered by E/B(th0) + next G half
                        phaseG(0)
                    phaseE(1, layer)
                    if not last:
                        phaseB(1)
                        phaseG(1)

            # ============= readout: logitsT = (v @ readout)^T, V-sharded =============
            if DO_READOUT:
                nvb = (VS + 127) // 128
                with nc.named_scope("readout"):
                    for vb in range(nvb):
                        m = min(128, VS - vb * 128)
                        rot = rop.tile([128, ND, 128], BF16, tag="ro", name=_nm("ro_"))
                        for dc in range(ND):
                            nc.sync.dma_start(
                                rot[:, dc, :m],
                                ro_d.ap()[dc * 128:(dc + 1) * 128,
                                          vb * 128:vb * 128 + m])
                        pl0 = pbp.tile([128, TH], F32, tag="pb", name=_nm("pb_"))
                        pl1 = pbp.tile([128, TH], F32, tag="pb", name=_nm("pb_"))
                        pls = [pl0, pl1]
                        for dc in range(ND):
                            for th in range(2):
                                for ns in range(2):
                                    nc.tensor.matmul(
                                        pls[th][:m, ns * 512:(ns + 1) * 512],
                                        rot[:, dc, :m],
                                        vpT[:, dc, th * TH + ns * 512:
                                            th * TH + (ns + 1) * 512],
                                        start=(dc == 0), stop=(dc == ND - 1),
                                        skip_group_check=True)
                        lo = lop.tile([128, T], BF16, tag="lo", name=_nm("lo_"))
                        nc.scalar.copy(lo[:m, 0:TH], pl0[:m])
                        nc.vector.tensor_copy(lo[:m, TH:T], pl1[:m])
                        nc.sync.dma_start(out_d.ap()[vb * 128:vb * 128 + m, :], lo[:m])

    nc.compile()
    return nc


_NC_CACHE = None


def _get_nc():
    global _NC_CACHE
    if _NC_CACHE is None:
        nc = bacc.Bacc("TRN2", target_bir_lowering=False, debug=False, num_devices=8)
        _NC_CACHE = build(nc)
    return _NC_CACHE


def _rope_tables():
    # match the jax reference: float32 angle computation, then bf16 cast
    import ml_dtypes
    inv_freq = (1.0 / (10000.0 ** (np.arange(0, K, 2, dtype=np.float32)
                                   / np.float32(K)))).astype(np.float32)
    t = np.arange(T, dtype=np.float32)
    freqs = (t[:, None] * inv_freq[None, :]).astype(np.float32)  # [T, K/2]
    cos = np.cos(freqs).astype(np.float32)
    sin = np.sin(freqs).astype(np.float32)
    # [K/2, T] -> [4, 128, 2, 1024] -> [8, 128, 1024] with index i*2+th
    def pack(a):
        aT = np.ascontiguousarray(a.T).reshape(4, 128, 2, TH)
        return np.ascontiguousarray(
            aT.transpose(0, 2, 1, 3).reshape(8, 128, TH)).astype(ml_dtypes.bfloat16)
    return pack(cos), pack(sin)


def kernel(input_, emb, pos, Dx, Dy, E, readout):
    import ml_dtypes
    BF = ml_dtypes.bfloat16
    input_ = np.asarray(input_)
    emb = np.ascontiguousarray(np.asarray(emb, dtype=np.float32))
    pos = np.ascontiguousarray(np.asarray(pos, dtype=np.float32))
    Dx = np.asarray(Dx, dtype=np.float32)
    Dy = np.asarray(Dy, dtype=np.float32)
    E = np.asarray(E, dtype=np.float32)
    readout = np.asarray(readout, dtype=np.float32)

    nc = _get_nc()
    cosb, sinb = _rope_tables()
    ro_bf = readout.astype(BF)

    in_maps = []
    for c in range(8):
        b, h = divmod(c, 4)
        in_maps.append({
            "tok": np.ascontiguousarray(input_[b].astype(np.int32)),
            "emb": emb,
            "posb": np.ascontiguousarray(pos.astype(BF)),
            "dxb": np.ascontiguousarray(Dx[h].astype(BF)),
            "dyb": np.ascontiguousarray(Dy[h].astype(BF)),
            "eb": np.ascontiguousarray(E[h * K:(h + 1) * K].astype(BF)),
            "rob": np.ascontiguousarray(ro_bf[:, h * VS:(h + 1) * VS]),
            "cosb": cosb,
            "sinb": sinb,
        })
    trace = os.environ.get("KRN_TRACE", "0") == "1"
    res = run_bass_kernel_spmd(nc, in_maps, list(range(8)), trace=trace)
    out = np.empty((B, T, V), dtype=np.float32)
    for c in range(8):
        b, h = divmod(c, 4)
        out[b, :, h * VS:(h + 1) * VS] = res.results[c]["logitsT"].astype(np.float32).T
    kernel._last_results = res
    return out
